# revision 4
# baseline (speedup 1.0000x reference)
"""Trainium2 Bass kernel for nn_BatchedGaussianRenderer.

Math: the per-pixel exponent of each gaussian is expanded as a 6-term
polynomial in centered pixel coordinates (x', y') = (x-63.5, y-63.5):

  expo(n, x, y) = f1*x'^2 + f2*x'y' + f3*y'^2 + f4*x' + f5*y' + f6
  image(x, y)   = sum_n exp(expo(n, x, y)),  then / max(image)

so the dense N x P evaluation is a K=6 matmul.  For fp32-grade accuracy on
the bf16 tensor engine, f and the pixel basis g are each split into 3 bf16
components and 6 cross products are kept (K=36, error ~2^-26 per term,
validated at 3.5e-6 absmax-rel vs an fp64 oracle).

Sharding: each of the 8 cores computes ALL 4096 gaussians' coefficients
(cheap, ~100 vector ops on [128, 32-block] layouts) and renders 16 image
rows (pixels x on partitions, gaussians streamed).  The ScalarEngine's
fused exp+row-sum (accum_out) produces the image directly; only a [1,1]
AllReduce(max) collective is needed for the final normalization.

Per-gaussian preprocessing uses the unnormalized-quaternion fold:
cov4D scales uniformly by nsq = |q1|^2 |q2|^2, which cancels everywhere
except eps -> eps*nsq and inv_cov/lambda -> *nsq, avoiding rsqrt entirely.
"""
import numpy as np
import ml_dtypes

import concourse.bass as bass
import concourse.bacc as bacc
import concourse.tile as tile
import concourse.mybir as mybir
from concourse import bass_utils

NG, H, W = 4096, 128, 128
ZOOM, EPS = 0.5, 1e-6
CX = CY = 63.5
SXY = (W - 1) / 2 * ZOOM          # 31.75
NCORES = 8
ROWS = H // NCORES                # 16 image rows per core
NB = NG // 128                    # 32 gaussian blocks (g = p*NB + b)
NSLOT = 6                         # (f-split, g-split) pairs
KP = NSLOT * 6                    # 36 K rows
dt = mybir.dt
AF = mybir.ActivationFunctionType
ALU = mybir.AluOpType
PI_2 = float(np.pi / 2)

# ---------------------------------------------------------------- host helpers

def _bf16(x):
    return np.asarray(x, np.float32).astype(ml_dtypes.bfloat16).astype(np.float32)


def _g_lhsT_for_core(core):
    """[KP, ROWS*128] bf16: per image row r (y = 16*core + r), cols 128r..128r+128
    hold the 36 K-rows of the pixel basis for the 128 x positions.
    K-row (s*6 + k): g-split j(s) of basis k, slots s = (hi,g0),(hi,g1),(hi,g2),
    (lo,g0),(lo,g1),(lo2,g0) -> j(s) = (0,1,2,0,1,0)."""
    j_of_s = (0, 1, 2, 0, 1, 0)
    out = np.zeros((KP, ROWS * 128), np.float32)
    x = np.arange(128, dtype=np.float64) - CX
    for r in range(ROWS):
        y = 16 * core + r - CY
        basis = np.stack([x * x, x * y, np.full(128, y * y), x,
                          np.full(128, y), np.ones(128)], 0)  # (6, 128)
        b32 = basis.astype(np.float32)
        g0 = _bf16(b32)
        g1 = _bf16(b32 - g0)
        g2 = _bf16(b32 - g0 - g1)
        gs = (g0, g1, g2)
        for s in range(NSLOT):
            for k in range(6):
                out[s * 6 + k, 128 * r:128 * (r + 1)] = gs[j_of_s[s]][k]
    return out.astype(ml_dtypes.bfloat16)


# L(q1) entry copies: L stored per-block as (i,k) slot = i*4+k, from rotor
# comps a=(r0,r4,r5,r6).  Each copy: (out_offset, out_stride, in_offset,
# in_stride, count, sign).
L_COPIES = [
    (0, 1, 0, 1, 1, 1.0),     # i0k0 <- r0
    (4, 4, 4, 1, 3, 1.0),     # (i1..i3)k0 <- r4,r5,r6
    (1, 12, 4, 1, 2, -1.0),   # i0k1<-r4-, i3k1<-r5-
    (5, 4, 0, 6, 2, 1.0),     # i1k1<-r0+, i2k1<-r6+
    (2, 4, 5, 1, 2, -1.0),    # i0k2<-r5-, i1k2<-r6-
    (10, 4, 0, 4, 2, 1.0),    # i2k2<-r0+, i3k2<-r4+
    (3, 1, 6, 1, 1, -1.0),    # i0k3<-r6-
    (11, 1, 4, 1, 1, -1.0),   # i2k3<-r4-
    (7, 1, 5, 1, 1, 1.0),     # i1k3<-r5+
    (15, 1, 0, 1, 1, 1.0),    # i3k3<-r0+
]
# R(conj q2) with q2 = (r7, -r1, -r2, -r3) folded in.  Stored (j,k) slot=j*4+k.
R_COPIES = [
    (0, 1, 7, 1, 1, 1.0),     # j0k0 <- r7
    (4, 4, 1, 1, 3, -1.0),    # (j1..j3)k0 <- -r1,-r2,-r3
    (1, 4, 1, 6, 2, 1.0),     # j0k1<-r1+, j1k1<-r7+
    (9, 1, 3, 1, 1, -1.0),    # j2k1<-r3-
    (13, 1, 2, 1, 1, 1.0),    # j3k1<-r2+
    (2, 4, 2, 1, 2, 1.0),     # j0k2<-r2+, j1k2<-r3+
    (10, 1, 7, 1, 1, 1.0),    # j2k2<-r7+
    (14, 1, 1, 1, 1, -1.0),   # j3k2<-r1-
    (3, 1, 3, 1, 1, 1.0),     # j0k3<-r3+
    (7, 1, 2, 1, 1, -1.0),    # j1k3<-r2-
    (11, 4, 1, 6, 2, 1.0),    # j2k3<-r1+, j3k3<-r7+
]


def _ap3(t, *dims):
    """View a 2D [128, X] tile slice as multi-dim via rearrange."""
    return t


def build_nc():
    nc = bacc.Bacc("TRN2", target_bir_lowering=False, debug=False,
                   num_devices=NCORES)
    f32, bf16 = dt.float32, dt.bfloat16

    means_in = nc.dram_tensor("means", [NG, 4], f32, kind="ExternalInput").ap()
    raws_in = nc.dram_tensor("raw_scales", [NG, 4], f32, kind="ExternalInput").ap()
    rot_in = nc.dram_tensor("rotors", [NG, 8], f32, kind="ExternalInput").ap()
    t_in = nc.dram_tensor("t_scalar", [1, 1], f32, kind="ExternalInput").ap()
    ang_in = nc.dram_tensor("angle", [1, 1], f32, kind="ExternalInput").ap()
    g_in = nc.dram_tensor("g_lhsT", [KP, ROWS * 128], bf16, kind="ExternalInput").ap()
    idb_in = nc.dram_tensor("ident_bf", [128, 128], bf16, kind="ExternalInput").ap()
    idf_in = nc.dram_tensor("ident_f32", [128, 128], f32, kind="ExternalInput").ap()
    ones_in = nc.dram_tensor("ones_row", [1, 128], f32, kind="ExternalInput").ap()
    out_t = nc.dram_tensor("out", [ROWS, W], f32, kind="ExternalOutput").ap()

    with tile.TileContext(nc) as tc:
        with (
            tc.tile_pool(name="sb", bufs=1) as sb,
            tc.tile_pool(name="dram", bufs=1, space="DRAM") as dram,
        ):
            # ---------------- phase 0: loads + angle scalars ----------------
            MEANS = sb.tile([128, NB * 4], f32)
            RAWS = sb.tile([128, NB * 4], f32)
            ROT = sb.tile([128, NB * 8], f32)
            nc.sync.dma_start(MEANS[:], means_in.rearrange("(p b) c -> p (b c)", p=128))
            nc.sync.dma_start(RAWS[:], raws_in.rearrange("(p b) c -> p (b c)", p=128))
            nc.sync.dma_start(ROT[:], rot_in.rearrange("(p b) c -> p (b c)", p=128))
            G_SB = sb.tile([KP, ROWS * 128], bf16)
            nc.sync.dma_start(G_SB[:], g_in[:])
            IDB = sb.tile([128, 128], bf16)
            nc.sync.dma_start(IDB[:], idb_in[:])
            IDF = sb.tile([128, 128], f32)
            nc.sync.dma_start(IDF[:], idf_in[:])
            ONES = sb.tile([1, 128], f32)
            nc.sync.dma_start(ONES[:], ones_in[:])
            T_A = sb.tile([1, 1], f32)
            nc.sync.dma_start(T_A[:], t_in[:])
            ANG = sb.tile([1, 1], f32)
            nc.sync.dma_start(ANG[:], ang_in[:])

            SINA = sb.tile([1, 1], f32)
            COSA = sb.tile([1, 1], f32)
            PIT = sb.tile([1, 1], f32)
            nc.vector.memset(PIT[:], PI_2)
            nc.scalar.activation(SINA[:], ANG[:], AF.Sin)
            nc.scalar.activation(COSA[:], ANG[:], AF.Sin, bias=PIT[:])

            # scalar vector: [sxc, sxs, A1, A2, A3, B1, B2, t]
            SCV = sb.tile([1, 8], f32)
            nc.vector.tensor_scalar_mul(SCV[:, 0:1], COSA[:], float(SXY))
            nc.vector.tensor_scalar_mul(SCV[:, 1:2], SINA[:], float(SXY))
            nc.vector.tensor_mul(SCV[:, 2:3], SCV[:, 0:1], SCV[:, 0:1])
            nc.vector.scalar_tensor_tensor(SCV[:, 3:4], SCV[:, 0:1], 2.0,
                                           SCV[:, 1:2], ALU.mult, ALU.mult)
            nc.vector.tensor_mul(SCV[:, 4:5], SCV[:, 1:2], SCV[:, 1:2])
            nc.vector.tensor_scalar_mul(SCV[:, 5:6], SCV[:, 0:1], float(SXY))
            nc.vector.tensor_scalar_mul(SCV[:, 6:7], SCV[:, 1:2], float(SXY))
            nc.vector.tensor_copy(SCV[:, 7:8], T_A[:])
            PB = None
            with tc.tile_pool(name="pp0", bufs=1, space="PSUM") as pp0:
                PBp = pp0.tile([128, 8], f32)
                nc.tensor.matmul(PBp[:], ONES[:], SCV[:], start=True, stop=True)
                SCB = sb.tile([128, 8], f32)
                nc.vector.tensor_copy(SCB[:], PBp[:])
            sxc_b, sxs_b = SCB[:, 0:1], SCB[:, 1:2]
            A1b, A2b, A3b = SCB[:, 2:3], SCB[:, 3:4], SCB[:, 4:5]
            B1b, B2b, tb = SCB[:, 5:6], SCB[:, 6:7], SCB[:, 7:8]

            # ---------------- phase 1: per-gaussian coefficients ----------------
            S2 = sb.tile([128, NB * 4], f32)
            nc.scalar.activation(S2[:], RAWS[:], AF.Exp, scale=2.0)

            SQ = sb.tile([128, NB * 8], f32)
            nc.vector.tensor_mul(SQ[:], ROT[:], ROT[:])
            sq = SQ[:].rearrange("p (b c) -> p b c", c=8)
            N1S = sb.tile([128, NB], f32)
            N2S = sb.tile([128, NB], f32)
            NSQ = sb.tile([128, NB], f32)
            # n1sq = sq0 + (sq4+sq5+sq6); n2sq = sq7 + (sq1+sq2+sq3)
            nc.vector.reduce_sum(N1S[:], sq[:, :, 4:7], axis=mybir.AxisListType.X)
            nc.vector.tensor_add(N1S[:], N1S[:], sq[:, :, 0])
            nc.vector.reduce_sum(N2S[:], sq[:, :, 1:4], axis=mybir.AxisListType.X)
            nc.vector.tensor_add(N2S[:], N2S[:], sq[:, :, 7])
            nc.vector.tensor_mul(NSQ[:], N1S[:], N2S[:])

            # L and R tiles (b, 16) via twisted copies
            LT = sb.tile([128, NB * 16], f32)
            RT = sb.tile([128, NB * 16], f32)
            rot3 = ROT[:].rearrange("p (b c) -> p b c", c=8)
            lt3 = LT[:].rearrange("p (b c) -> p b c", c=16)
            rt3 = RT[:].rearrange("p (b c) -> p b c", c=16)
            for dst, copies in ((lt3, L_COPIES), (rt3, R_COPIES)):
                for (oo, os_, io, is_, cnt, sign) in copies:
                    out_ap = dst[:, :, oo::os_][:, :, :cnt] if cnt > 1 else dst[:, :, oo:oo + 1]
                    if is_ > 0:
                        in_ap = rot3[:, :, io::is_][:, :, :cnt] if cnt > 1 else rot3[:, :, io:io + 1]
                    else:
                        # negative stride: build via explicit AP arithmetic
                        in_ap = rot3[:, :, io::is_][:, :, :cnt]
                    nc.any.tensor_scalar_mul(out_ap, in_ap, sign)

            # P64 = L (b,i,bc j,k) * R (b,bc i,j,k); reduce k -> R4 (b, i*4+j)
            P64 = sb.tile([128, NB * 64], f32)
            lt4 = LT[:].rearrange("p (b i k) -> p b i k", i=4, k=4)
            rt4 = RT[:].rearrange("p (b j k) -> p b j k", j=4, k=4)
            p5 = P64[:].rearrange("p (b i j k) -> p b i j k", i=4, j=4, k=4)
            nc.vector.tensor_mul(
                p5,
                lt4.unsqueeze(3).broadcast_to([128, NB, 4, 4, 4]),
                rt4.unsqueeze(2).broadcast_to([128, NB, 4, 4, 4]))
            R4 = sb.tile([128, NB * 16], f32)
            nc.vector.reduce_sum(
                R4[:].rearrange("p (b e) -> p b e", e=16),
                P64[:].rearrange("p (e k) -> p e k", k=4),
                axis=mybir.AxisListType.X)

            # M = R4 * s2[j] ; C64 = M (b,i,bc k,j) * R4 (b,bc i,k,j); reduce j
            M = sb.tile([128, NB * 16], f32)
            r44 = R4[:].rearrange("p (b i j) -> p b i j", i=4, j=4)
            s23 = S2[:].rearrange("p (b c) -> p b c", c=4)
            m4 = M[:].rearrange("p (b i j) -> p b i j", i=4, j=4)
            nc.vector.tensor_mul(
                m4, r44, s23.unsqueeze(2).broadcast_to([128, NB, 4, 4]))
            C64 = sb.tile([128, NB * 64], f32)
            c5 = C64[:].rearrange("p (b i k j) -> p b i k j", i=4, k=4, j=4)
            nc.vector.tensor_mul(
                c5,
                m4.unsqueeze(3).broadcast_to([128, NB, 4, 4, 4]),
                r44.unsqueeze(2).broadcast_to([128, NB, 4, 4, 4]))
            C16 = sb.tile([128, NB * 16], f32)
            nc.vector.reduce_sum(
                C16[:].rearrange("p (b e) -> p b e", e=16),
                C64[:].rearrange("p (e j) -> p e j", j=4),
                axis=mybir.AxisListType.X)
            c16 = C16[:].rearrange("p (b e) -> p b e", e=16)

            # scalar rows
            EPN = sb.tile([128, NB], f32)
            nc.vector.tensor_scalar_mul(EPN[:], NSQ[:], float(EPS))
            WP = sb.tile([128, NB], f32)
            nc.vector.tensor_max(WP[:], c16[:, :, 15], EPN[:])
            IW = sb.tile([128, NB], f32)
            nc.vector.reciprocal(IW[:], WP[:])
            means3 = MEANS[:].rearrange("p (b c) -> p b c", c=4)
            TD = sb.tile([128, NB], f32)
            nc.scalar.activation(TD[:], means3[:, :, 3], AF.Identity,
                                 bias=tb, scale=-1.0)
            TDW = sb.tile([128, NB], f32)
            nc.vector.tensor_mul(TDW[:], TD[:], IW[:])
            W1 = sb.tile([128, NB], f32)
            nc.vector.tensor_mul(W1[:], NSQ[:], IW[:])
            Z3 = sb.tile([128, NB], f32)
            nc.vector.tensor_mul(Z3[:], W1[:], TD[:])

            # VV9, cov3 (3x3, (i,k) slots)
            VV9 = sb.tile([128, NB * 9], f32)
            vv3 = VV9[:].rearrange("p (b i k) -> p b i k", i=3, k=3)
            v_i = c16[:, :, 3::4][:, :, 0:3]     # (p, b, 3) = cov4[(i,3)]
            nc.vector.tensor_mul(
                vv3,
                v_i.unsqueeze(3).broadcast_to([128, NB, 3, 3]),
                v_i.unsqueeze(2).broadcast_to([128, NB, 3, 3]))
            CV3 = sb.tile([128, NB * 9], f32)
            cv3 = CV3[:].rearrange("p (b e) -> p b e", e=9)
            iw_b9 = IW[:].rearrange("p b -> p b").unsqueeze(2).broadcast_to([128, NB, 9])
            vv9f = VV9[:].rearrange("p (b e) -> p b e", e=9)
            nc.vector.tensor_mul(cv3, vv9f, iw_b9)
            u9 = c16.rearrange("p b (i k) -> p b i k", i=4)[:, :, 0:3, 0:3]
            nc.vector.tensor_sub(CV3[:].rearrange("p (b i k) -> p b i k", i=3, k=3),
                                 u9, CV3[:].rearrange("p (b i k) -> p b i k", i=3, k=3))

            # mu3 = mean_xyz + V * tdw
            MU3 = sb.tile([128, NB * 3], f32)
            mu33 = MU3[:].rearrange("p (b c) -> p b c", c=3)
            tdw_b3 = TDW[:].rearrange("p b -> p b").unsqueeze(2).broadcast_to([128, NB, 3])
            nc.vector.tensor_mul(mu33, v_i, tdw_b3)
            nc.vector.tensor_add(mu33, mu33, means3[:, :, 0:3])

            # projection
            MX = sb.tile([128, NB], f32)
            MY = sb.tile([128, NB], f32)
            TMP = sb.tile([128, NB], f32)
            TMP2 = sb.tile([128, NB], f32)
            nc.vector.tensor_scalar(TMP[:], mu33[:, :, 2], sxs_b, None, ALU.mult)
            nc.vector.scalar_tensor_tensor(MX[:], mu33[:, :, 0], sxc_b, TMP[:],
                                           ALU.mult, ALU.add)
            nc.vector.tensor_scalar_mul(MY[:], mu33[:, :, 1], float(SXY))

            cv3e = CV3[:].rearrange("p (b e) -> p b e", e=9)
            AE = sb.tile([128, NB], f32)
            BE = sb.tile([128, NB], f32)
            DE = sb.tile([128, NB], f32)
            # a' = A1*c00 + A2*c02 + A3*c22  (+ epsn)
            nc.vector.tensor_scalar(TMP[:], cv3e[:, :, 8], A3b, None, ALU.mult)
            nc.vector.scalar_tensor_tensor(TMP[:], cv3e[:, :, 2], A2b, TMP[:],
                                           ALU.mult, ALU.add)
            nc.vector.scalar_tensor_tensor(AE[:], cv3e[:, :, 0], A1b, TMP[:],
                                           ALU.mult, ALU.add)
            nc.vector.tensor_add(AE[:], AE[:], EPN[:])
            # b' = B1*c01 + B2*c12
            nc.vector.tensor_scalar(TMP[:], cv3e[:, :, 5], B2b, None, ALU.mult)
            nc.vector.scalar_tensor_tensor(BE[:], cv3e[:, :, 1], B1b, TMP[:],
                                           ALU.mult, ALU.add)
            # d' = SXY^2 * c11 (+ epsn)
            nc.vector.tensor_scalar_mul(DE[:], cv3e[:, :, 4], float(SXY * SXY))
            nc.vector.tensor_add(DE[:], DE[:], EPN[:])

            DET = sb.tile([128, NB], f32)
            nc.vector.tensor_mul(DET[:], AE[:], DE[:])
            nc.vector.tensor_mul(TMP[:], BE[:], BE[:])
            nc.vector.tensor_sub(DET[:], DET[:], TMP[:])
            IDN = sb.tile([128, NB], f32)
            nc.vector.reciprocal(IDN[:], DET[:])
            nc.vector.tensor_mul(IDN[:], IDN[:], NSQ[:])
            IA = sb.tile([128, NB], f32)
            ID_ = sb.tile([128, NB], f32)
            F2T = sb.tile([128, NB], f32)
            nc.vector.tensor_mul(IA[:], DE[:], IDN[:])
            nc.vector.tensor_mul(F2T[:], BE[:], IDN[:])   # f2 = -ib = +b'*idn
            nc.vector.tensor_mul(ID_[:], AE[:], IDN[:])

            F6 = sb.tile([128, NB * 6], f32)
            f63 = F6[:].rearrange("p (b k) -> p b k", k=6)
            nc.vector.tensor_scalar_mul(f63[:, :, 0], IA[:], -0.5)
            nc.vector.tensor_copy(f63[:, :, 1], F2T[:])
            nc.vector.tensor_scalar_mul(f63[:, :, 2], ID_[:], -0.5)
            # f4 = ia*mx - f2*my ; f5 = id*my - f2*mx
            nc.vector.tensor_mul(TMP[:], IA[:], MX[:])
            nc.vector.tensor_mul(TMP2[:], F2T[:], MY[:])
            nc.vector.tensor_sub(f63[:, :, 3], TMP[:], TMP2[:])
            nc.vector.tensor_mul(TMP[:], ID_[:], MY[:])
            nc.vector.tensor_mul(TMP2[:], F2T[:], MX[:])
            nc.vector.tensor_sub(f63[:, :, 4], TMP[:], TMP2[:])
            # f6 = -0.5*(mx*f4 + my*f5 + z3*td)
            nc.vector.tensor_mul(TMP[:], MX[:], f63[:, :, 3])
            nc.vector.tensor_mul(TMP2[:], MY[:], f63[:, :, 4])
            nc.vector.tensor_add(TMP[:], TMP[:], TMP2[:])
            nc.vector.tensor_mul(TMP2[:], Z3[:], TD[:])
            nc.vector.tensor_add(TMP[:], TMP[:], TMP2[:])
            nc.vector.tensor_scalar_mul(f63[:, :, 5], TMP[:], -0.5)

            # bf16 splits -> F36 (b, s, k), s-groups: hi,hi,hi,lo,lo,lo2
            F36 = sb.tile([128, NB * KP], bf16)
            f364 = F36[:].rearrange("p (b s k) -> p b s k", s=NSLOT, k=6)
            R1 = sb.tile([128, NB * 6], f32)
            R2 = sb.tile([128, NB * 6], f32)
            r13 = R1[:].rearrange("p (b k) -> p b k", k=6)
            r23 = R2[:].rearrange("p (b k) -> p b k", k=6)
            nc.any.tensor_copy(f364[:, :, 0, :], f63)
            nc.any.tensor_copy(f364[:, :, 1, :], f364[:, :, 0, :])
            nc.any.tensor_copy(f364[:, :, 2, :], f364[:, :, 0, :])
            nc.vector.tensor_sub(r13, f63, f364[:, :, 0, :])
            nc.any.tensor_copy(f364[:, :, 3, :], r13)
            nc.any.tensor_copy(f364[:, :, 4, :], f364[:, :, 3, :])
            nc.vector.tensor_sub(r23, r13, f364[:, :, 3, :])
            nc.any.tensor_copy(f364[:, :, 5, :], r23)

            # transposes: F36 block b [128, 36] -> FSTACK[:, 128b:128b+128]
            FSTACK = sb.tile([KP, NG], bf16)
            with tc.tile_pool(name="pp", bufs=2, space="PSUM") as pp:
                for q in range(NB // 4):
                    TP = pp.tile([KP, 512], bf16)
                    for c in range(4):
                        b = 4 * q + c
                        nc.tensor.transpose(TP[:, 128 * c:128 * (c + 1)],
                                            F36[:, KP * b:KP * (b + 1)], IDB[:])
                    nc.vector.tensor_copy(FSTACK[:, 512 * q:512 * (q + 1)], TP[:])

            # ---------------- phase 2: dense render ----------------
            ACC = sb.tile([128, 2 * ROWS], f32)
            with tc.tile_pool(name="dp", bufs=2, space="PSUM") as dp:
                for r in range(ROWS):
                    for h2 in range(2):
                        PT = dp.tile([128, 2048], dt.float32, tag="pt")
                        for s in range(4):
                            nc.tensor.matmul(
                                PT[:, 512 * s:512 * (s + 1)],
                                G_SB[:, 128 * r:128 * (r + 1)],
                                FSTACK[:, 2048 * h2 + 512 * s:2048 * h2 + 512 * (s + 1)],
                                start=True, stop=True)
                        col = 2 * r + h2
                        nc.scalar.activation(PT[:], PT[:], AF.Exp,
                                             accum_out=ACC[:, col:col + 1])

            IMG = sb.tile([128, ROWS], f32)
            acc3 = ACC[:].rearrange("p (r h) -> p r h", h=2)
            nc.vector.tensor_add(IMG[:].rearrange("p r -> p r"),
                                 acc3[:, :, 0], acc3[:, :, 1])

            # ---------------- phase 3: global max + normalize ----------------
            RM = sb.tile([128, 1], f32)
            nc.vector.reduce_max(RM[:], IMG[:], axis=mybir.AxisListType.X)
            with tc.tile_pool(name="tp", bufs=1, space="PSUM") as tp:
                RMTp = tp.tile([1, 128], f32)
                nc.tensor.transpose(RMTp[:], RM[:], IDF[:])
                RMT = sb.tile([1, 128], f32)
                nc.vector.tensor_copy(RMT[:], RMTp[:])
                LMAX = sb.tile([1, 1], f32)
                nc.vector.reduce_max(LMAX[:], RMT[:], axis=mybir.AxisListType.X)

                cin = dram.tile([1, 1], f32)
                cout = dram.tile([1, 1], f32)
                nc.sync.dma_start(cin[:], LMAX[:])
                nc.gpsimd.collective_compute(
                    "AllReduce", ALU.max,
                    replica_groups=[list(range(NCORES))],
                    ins=[cin[:].opt()], outs=[cout[:].opt()])
                GM = sb.tile([1, 1], f32)
                nc.sync.dma_start(GM[:], cout[:])
                nc.vector.tensor_scalar_max(GM[:], GM[:], float(EPS))
                RI = sb.tile([1, 1], f32)
                nc.vector.reciprocal(RI[:], GM[:])
                RIBp = tp.tile([128, 1], f32)
                nc.tensor.matmul(RIBp[:], ONES[:], RI[:], start=True, stop=True)
                RIB = sb.tile([128, 1], f32)
                nc.vector.tensor_copy(RIB[:], RIBp[:])
                nc.vector.tensor_scalar(IMG[:], IMG[:], RIB[:], None, ALU.mult)

                OTp = tp.tile([ROWS, 128], f32)
                nc.tensor.transpose(OTp[:], IMG[:], IDF[:])
                OT = sb.tile([ROWS, 128], f32)
                nc.vector.tensor_copy(OT[:], OTp[:])
                nc.sync.dma_start(out_t[:], OT[:])

    nc.compile()
    return nc


_NC_CACHE = {}


def _get_nc():
    if "nc" not in _NC_CACHE:
        _NC_CACHE["nc"] = build_nc()
    return _NC_CACHE["nc"]


def _make_in_maps(means, raw_scales, rotors, t, angle):
    means = np.ascontiguousarray(np.asarray(means, np.float32))
    raw_scales = np.ascontiguousarray(np.asarray(raw_scales, np.float32))
    rotors = np.ascontiguousarray(np.asarray(rotors, np.float32))
    t_arr = np.array([[np.float32(t)]], np.float32)
    a_arr = np.array([[np.float32(angle)]], np.float32)
    idb = np.eye(128, dtype=np.float32).astype(ml_dtypes.bfloat16)
    idf = np.eye(128, dtype=np.float32)
    ones = np.ones((1, 128), np.float32)
    in_maps = []
    for c in range(NCORES):
        in_maps.append({
            "means": means, "raw_scales": raw_scales, "rotors": rotors,
            "t_scalar": t_arr, "angle": a_arr,
            "g_lhsT": np.ascontiguousarray(_g_lhsT_for_core(c)),
            "ident_bf": idb, "ident_f32": idf, "ones_row": ones,
        })
    return in_maps


def run(means, raw_scales, rotors, t, angle, trace=False):
    """Returns (image [128,128] fp32, BassKernelResults)."""
    nc = _get_nc()
    in_maps = _make_in_maps(means, raw_scales, rotors, t, angle)
    res = bass_utils.run_bass_kernel_spmd(
        nc, in_maps, core_ids=list(range(NCORES)), trace=trace)
    img = np.concatenate([res.results[c]["out"] for c in range(NCORES)], axis=0)
    return img.astype(np.float32), res


def kernel(**inputs):
    img, _ = run(inputs["means"], inputs["raw_scales"], inputs["rotors"],
                 inputs["t"], inputs["angle"])
    return img
